# revision 20
# baseline (speedup 1.0000x reference)
"""Trainium2 Bass kernel for CombinedVectorField (CFG vector field + exact
Jacobian-trace divergence).

Math: with u = tanh(x@W1x + h@W1h + b1'), b1' = b1 + t*W1[256],
  v(x,h)  = u @ W2 + b2
  div(x,h)= sum_k (1-u_k^2) c_k = d0 - (u*u) @ c,   c_k = sum_i W1x[i,k] W2[k,i]
Output = concat[(1-gs)*v_null + gs*v_h, (1-gs)*div_null + gs*div_h].

Sharding: pure data parallel - each of the 8 cores takes 512 batch rows
(both guidance branches), weights replicated, feature-major layouts.

Schedule notes:
- The first-needed bytes (row-half-a activations + chunk-0 weights +
  biases) are split across BOTH HWDGE rings so they land ~2x sooner;
  inB/inC follow in need order; inD (W2) goes via SWDGE, its issue
  delayed behind dummy memsets so it doesn't steal early bandwidth. A
  dummy activation right after inA1's issue pulls the ~1.4us ACT table
  load ahead of the first real tanh.
- Each (chunk, half) L1 output is one [128,512] PSUM bank holding both
  guidance branches side by side -> one tanh ACTIVATE per group with a
  per-partition f32 chunk bias (bf16-shipped, converted once on DVE).
- The tanh chain runs [c0a c1a c2a c0b c1b c2b c3b c3a]: the b-half
  accumulator (feeding the VO_b DMA) closes one tanh early, so the only
  thing tailing the final tanh is the tiny divergence-partials DMA.
- Divergence matmuls are column-tiled 4-way (tile_position=(0,32c)):
  four chunk partials run concurrently into distinct partitions of one
  bank and ship as 4 bf16 rows summed on the host with d0.
- PE prewarm (right-sized to end as inA lands) keeps the PE-HAM
  activity window busy from kernel entry so real matmuls reach 2.4 GHz
  sooner.
"""
import sys

sys.path.insert(0, "/opt/trn_rl_repo")

import ml_dtypes
import numpy as np

import concourse.bass as bass
import concourse.tile as tile
from concourse import bacc, mybir
from concourse.bass_utils import run_bass_kernel_spmd
from concourse.vector_clock import ScopedClock


class _TrimTileContext(tile.TileContext):
    """TileContext with the final all-engine barrier dropped from the
    teardown and the mid barrier reduced to sem-only (no per-engine
    drains). The head drain still waits for every semaphore (incl.
    output-DMA completion) and semaphores are still cleared for the next
    execution; only the trailing barrier (nothing executes after it) is
    elided."""

    def _drain_and_barrier(self, tick_clock, wait_clock):
        drain_inst = self.nc.sync.drain()
        wait_clock.add_sem_waits(
            drain_inst.ins, ScopedClock({None: tick_clock.global_clock})
        )
        self.nc.all_engine_barrier(sem_only=True)
        popped = self.nc._tile_sem_poison_stack.pop()
        assert popped is self._sem_poison
        self.nc.clear_and_free_semaphores(list(self.sems.allocated().values()))


class _FastBacc(bacc.Bacc):
    """Bacc whose constructor-time all-engine barrier (after the const-tile
    memsets) is sem-only - the per-engine drains there cost ~1us of kernel
    head time and order nothing we rely on beyond the memsets, which the
    event-semaphore barrier already orders."""

    def all_engine_barrier(self, *, sem_only: bool = False):
        super().all_engine_barrier(sem_only=True)

F32 = mybir.dt.float32
BF16 = mybir.dt.bfloat16
AF = mybir.ActivationFunctionType
ALU = mybir.AluOpType

N_CORES = 8
B = 4096
DIM_X = 128
DIM_H = 128
HIDDEN = 512
R = B // N_CORES          # rows per core
HR = R // 2               # rows per half
NCH = HIDDEN // 128       # hidden chunks
NE1 = HR + 512 + 5        # E1 cols: xa|w1x_all|b1'(4)|b2
NE2 = 2 * HR + 512         # E2 cols: ha|hna|w1h_all

_NC_CACHE = None


def _build():
    nc = _FastBacc("TRN2", target_bir_lowering=False, debug=False,
                   enable_asserts=False, monotonic_sem_count=0)

    inE1 = nc.dram_tensor("inE1", [128, NE1], BF16, kind="ExternalInput")
    inE2 = nc.dram_tensor("inE2", [128, NE2], BF16, kind="ExternalInput")
    inC = nc.dram_tensor("inC", [128, 3 * HR], BF16, kind="ExternalInput")
    inD = nc.dram_tensor("inD", [128, 2 * NCH * DIM_X + 2 * NCH], BF16,
                         kind="ExternalInput")

    VO = nc.dram_tensor("VO", [DIM_X, R], BF16, kind="ExternalOutput")
    DO = nc.dram_tensor("DO", [NCH, R], BF16, kind="ExternalOutput")

    with _TrimTileContext(nc) as tc:
        with tc.tile_pool(name="cst", bufs=1) as cst, \
             tc.tile_pool(name="act", bufs=8) as actp, \
             tc.tile_pool(name="out", bufs=1) as outp, \
             tc.tile_pool(name="psg", bufs=4, space="PSUM") as psg, \
             tc.tile_pool(name="psv", bufs=1, space="PSUM") as psv:
            # PE prewarm: 14 cold 256-col matmuls ~= 3us of PE busy,
            # sized to end right as the first blob lands.
            wrm = cst.tile([128, 256], BF16)
            nc.gpsimd.memset(wrm[:], 0.0)
            pwarm = psv.tile([128, 256], F32, tag="pd")
            for _ in range(14):
                nc.tensor.matmul(pwarm[:], wrm[:, 0:128], wrm[:],
                                 start=True, stop=True, skip_group_check=True)

            at = cst.tile([128, NE1], BF16)
            nc.scalar.dma_start(out=at[:], in_=inE1[:])
            a2t = cst.tile([128, NE2], BF16)
            nc.sync.dma_start(out=a2t[:], in_=inE2[:])
            # dummy activation: pulls the ACT table load ahead, overlapping
            # E1's transfer instead of delaying the first real tanh.
            warmact = outp.tile([128, 1], F32)
            nc.scalar.activation(warmact[:], nc.const_aps.aps[(F32, 0.0)],
                                 AF.Tanh, bias=0.0, scale=1.0)
            # ring FIFO enforces need order: inD behind E1 on scalar,
            # inC behind E2 on sync. No SWDGE input at all.
            dt = cst.tile([128, 2 * NCH * DIM_X + 2 * NCH], BF16)
            nc.scalar.dma_start(out=dt[:], in_=inD[:])
            ct = cst.tile([128, 3 * HR], BF16)
            nc.sync.dma_start(out=ct[:], in_=inC[:])

            # bf16 -> f32 bias conversion (one cheap DVE op once E1 lands)
            f32aux = cst.tile([128, 5], F32)
            nc.vector.tensor_copy(f32aux[:], at[:, HR + 512:NE1])

            xa = at[:, 0:HR]
            ha = a2t[:, 0:HR]
            hna = a2t[:, HR:2 * HR]
            xb = ct[:, 0 * HR:1 * HR]
            hb = ct[:, 1 * HR:2 * HR]
            hnb = ct[:, 2 * HR:3 * HR]

            def w1x(c):
                return at[:, HR + 128 * c:HR + 128 * (c + 1)]

            def w1h(c):
                return a2t[:, 2 * HR + 128 * c:2 * HR + 128 * (c + 1)]

            def w2c(c, br):
                off = br * NCH * DIM_X
                return dt[:, off + 128 * c:off + 128 * (c + 1)]

            def cmc(c, br):
                off = 2 * NCH * DIM_X + br * NCH
                return dt[:, off + c:off + c + 1]

            pva = psv.tile([128, HR], F32, tag="pva")
            pvb = psv.tile([128, HR], F32, tag="pvb")
            pd = psv.tile([128, R], F32, tag="pd")

            ut = {}
            u2t = {}
            gt = {}

            def l1(c, half, x_, h_, hn_):
                g = psg.tile([128, 2 * HR], F32, tag="g")
                gt[(c, half)] = g
                nc.tensor.matmul(g[:, 0:HR], w1x(c), x_, start=True, stop=False)
                nc.tensor.matmul(g[:, HR:2 * HR], w1x(c), x_, start=False, stop=False)
                nc.tensor.matmul(g[:, 0:HR], w1h(c), h_, start=False, stop=False)
                nc.tensor.matmul(g[:, HR:2 * HR], w1h(c), hn_, start=False, stop=True)

            def act(c, half, defer_u2=False):
                u = actp.tile([128, 2 * HR], BF16, tag="u")
                ut[(c, half)] = u
                nc.scalar.activation(u[:], gt[(c, half)][:], AF.Tanh,
                                     bias=f32aux[:, c:c + 1], scale=1.0)
                if not defer_u2:
                    emit_u2(c, half)

            def emit_u2(c, half):
                u2 = actp.tile([128, 2 * HR], BF16, tag="u2")
                u2t[(c, half)] = u2
                nc.vector.tensor_tensor(u2[:], ut[(c, half)][:], ut[(c, half)][:],
                                        op=ALU.mult)

            def l2(c, half, pv, first, last):
                u = ut[(c, half)]
                nc.tensor.matmul(pv[:], w2c(c, 0), u[:, 0:HR],
                                 start=first, stop=False)
                nc.tensor.matmul(pv[:], w2c(c, 1), u[:, HR:2 * HR],
                                 start=False, stop=last)

            def pdiv(br, half, first, last):
                # 4 chunk partials run concurrently on distinct col groups,
                # landing at partitions {0,32,64,96} of the pd bank.
                cs = slice(0, HR) if half == 0 else slice(HR, R)
                us = slice(0, HR) if br == 0 else slice(HR, 2 * HR)
                for c in range(NCH):
                    nc.tensor.matmul(pd[32 * c:32 * c + 1, cs], cmc(c, br),
                                     u2t[(c, half)][:, us],
                                     start=(first and c == 0),
                                     stop=(last and c == NCH - 1),
                                     tile_position=(0, 32 * c))

            # chain order: b-half accumulator closes at tanh #7, a-half at
            # tanh #8 - only the small divergence DMA tails the last tanh.
            CHAIN = [(0, 0), (1, 0), (2, 0), (0, 1), (1, 1), (2, 1), (3, 1), (3, 0)]
            for c, half in CHAIN:
                if half == 0:
                    l1(c, 0, xa, ha, hna)
                else:
                    l1(c, 1, xb, hb, hnb)
                # the final chunk's u*u is emitted after voutb's DVE op so
                # the VO_b evacuation isn't queued behind it
                act(c, half, defer_u2=((c, half) == (3, 0)))

            l2(0, 1, pvb, True, False)
            l2(1, 1, pvb, False, False)
            l2(2, 1, pvb, False, False)
            l2(0, 0, pva, True, False)
            l2(1, 0, pva, False, False)
            l2(2, 0, pva, False, False)
            l2(3, 1, pvb, False, True)
            # pvb closes here: evacuate v half-b on DVE (TS, f32 b2) so it
            # isn't queued behind the final tanh on ACT.
            voutb = outp.tile([128, HR], BF16)
            nc.vector.tensor_scalar(voutb[:], pvb[:], f32aux[:, 4:5], None,
                                    op0=ALU.add)
            nc.sync.dma_start(out=VO[:, HR:R], in_=voutb[:])
            pdiv(0, 1, True, False)
            pdiv(1, 1, False, False)
            pdc = outp.tile([128, R], BF16)
            nc.vector.tensor_copy(pdc[:, HR:R], pd[:, HR:R])
            u2f = actp.tile([128, 2 * HR], BF16, tag="u2")
            u2t[(3, 0)] = u2f
            nc.scalar.activation(u2f[:], ut[(3, 0)][:], AF.Square,
                                 bias=0.0, scale=1.0)
            l2(3, 0, pva, False, True)
            pdiv(0, 0, False, False)
            pdiv(1, 0, False, True)

            # v half-a on ACT (free right after the last tanh), out on the
            # scalar ring; divergence partials: one bank copy -> 4-row DMA
            # on sync, summed on the host.
            vouta = outp.tile([128, HR], BF16)
            nc.scalar.activation(vouta[:], pva[:], AF.Identity,
                                 bias=f32aux[:, 4:5], scale=1.0)
            nc.scalar.dma_start(out=VO[:, 0:HR], in_=vouta[:])
            nc.vector.tensor_copy(pdc[:, 0:HR], pd[:, 0:HR])
            nc.sync.dma_start(out=DO[:], in_=pdc[0:97:32, :])
    nc.compile()
    return nc


def _get_nc():
    global _NC_CACHE
    if _NC_CACHE is None:
        _NC_CACHE = _build()
    return _NC_CACHE


def _prep_in_maps(state, h, h_null, t, guidance_scale, W1, b1, W2, b2):
    f32 = np.float32
    bf = ml_dtypes.bfloat16
    xTf = state[:, :DIM_X].T.astype(bf)                            # (128, B)
    hTf = h.T.astype(bf)
    hnTf = h_null.T.astype(bf)
    w1xf = W1[:DIM_X].astype(bf)                                   # (128, 512)
    w1hf = W1[DIM_X:DIM_X + DIM_H].astype(bf)
    b1p = (b1.astype(f32) + t.astype(f32)[0] * W1[DIM_X + DIM_H].astype(f32))
    w2r = W2.astype(f32).reshape(NCH, 128, DIM_X).transpose(1, 0, 2).reshape(128, NCH * DIM_X)
    cvec = (W1[:DIM_X].astype(np.float64) * W2.astype(np.float64).T).sum(0)  # (512,)
    d0 = float(cvec.sum())
    cmatf = cvec.reshape(NCH, 128).T.astype(f32)                   # (128, NCH)
    gs = float(guidance_scale.astype(f32)[0])
    inD = np.ascontiguousarray(
        np.concatenate([gs * w2r, (1.0 - gs) * w2r,
                        -gs * cmatf, -(1.0 - gs) * cmatf], axis=1).astype(bf))


    auxf = np.zeros((128, 5), f32)
    auxf[:, 0:4] = b1p.reshape(NCH, 128).T
    auxf[:, 4] = b2.astype(f32)
    auxbf = auxf.astype(bf)

    in_maps = []
    for i in range(N_CORES):
        sl_a = slice(i * R, i * R + HR)
        sl_b = slice(i * R + HR, (i + 1) * R)
        in_maps.append({
            "inE1": np.ascontiguousarray(
                np.concatenate([xTf[:, sl_a], w1xf, auxbf], axis=1)),
            "inE2": np.ascontiguousarray(
                np.concatenate([hTf[:, sl_a], hnTf[:, sl_a], w1hf], axis=1)),
            "inC": np.ascontiguousarray(
                np.concatenate([xTf[:, sl_b], hTf[:, sl_b], hnTf[:, sl_b]],
                               axis=1)),
            "inD": inD,
        })
    return in_maps, d0


def kernel(state, h, h_null, t, guidance_scale, W1, b1, W2, b2, _trace=False):
    nc = _get_nc()
    in_maps, d0 = _prep_in_maps(state, h, h_null, t, guidance_scale,
                                W1, b1, W2, b2)
    res = run_bass_kernel_spmd(nc, in_maps, list(range(N_CORES)), trace=_trace)
    out = np.empty((B, DIM_X + 1), np.float32)
    for i in range(N_CORES):
        sl = slice(i * R, (i + 1) * R)
        out[sl, :DIM_X] = res.results[i]["VO"].astype(np.float32).T
        out[sl, DIM_X] = res.results[i]["DO"].astype(np.float32).sum(0) + d0
    if _trace:
        return out, res
    return out
